# revision 30
# baseline (speedup 1.0000x reference)
"""Trainium2 Bass kernel for nn_DecoderCrossAttention.

Reference computation (per voxel v, batch b):
    q = Wq x_v + bq                        (x = decoder_features, [C])
    k_j = Wk y_jv + bk, v_j = Wv y_jv + bv (y = skip features, COND=4 frames)
    s_j[h] = <q_h, k_jh> / sqrt(DH)        (NH=8 heads of DH=16)
    attn = softmax_j(s)                    (over the 4 conditioning frames)
    o = Wo (sum_j attn_j * v_j) + bo + x_v
    out = GroupNorm8(o) * gamma + beta     (stats over (C/G, H, W, D) per batch)

Algebraic folds:
  * bk shifts all 4 logits of each softmax equally -> dropped entirely.
  * sum_j attn_j == 1, so bv contributes Wo bv to every voxel ->
    folded into bo2 = Wo bv + bo (computed once on device).

Strategy (8 NeuronCores, data-parallel over H):
  * Each core gets H-slice of 4 planes: 2*4*32*32 = 8192 voxels.
  * Feature-major layout [C=128 partitions, voxels in free dim], 512-voxel
    tiles, 4-deep software pipeline (front / softmax / attn-apply / out).
  * All projections are PE matmuls in float32r (full rate at N=512).
  * Per-head score reduction and the softmax broadcast (8 head rows -> 128
    channels) are PE matmuls against 0/1 masks built in-kernel.
  * Softmax tail runs in bf16 (exp out, 1/Z, attn weights, broadcast
    matmul) - masks are exact 0/1 in bf16, 2x DVE rate on etsb.
  * V parks in PSUM; attn*V is a Pool tensor_tensor reading both operands
    from PSUM - no PSUM->SBUF moves for V at all.
  * Residual + bo2 + per-channel GN sums fused in one DVE stt; squares
    accumulated on Act.
  * GroupNorm stats cross over cores via AllGather (15us) + local strided
    reduce instead of AllReduce (28us); GN rescale is Act Identity with
    per-partition scale/bias vectors.

The walrus build here accepts only ONE sync wait per instruction; Tile
attaches many.  split_waits() hoists extras onto standalone EventSemaphore
instructions post-scheduling.
"""

import sys

if "/opt/trn_rl_repo" not in sys.path:
    sys.path.insert(0, "/opt/trn_rl_repo")

import numpy as np

B, COND, C, H, W, D = 2, 4, 128, 32, 32, 32
NH, DH, G = 8, 16, 8
EPS = 1e-5
NCORES = 8
HS = H // NCORES          # 4 H-planes per core
NVOX = HS * W * D         # 4096 voxels per batch per core
NT = 512                  # voxels per tile
NTILES = NVOX // NT       # 8 tiles per batch
N_GROUP = (C // G) * H * W * D   # elements per (batch, group) for GN stats

_CACHE = {}


def _split_waits(nc):
    """Hoist extra sync waits onto standalone EventSemaphore instructions."""
    from concourse import mybir
    import bass_rust

    n_split = 0
    for func in nc.m.functions:
        for blk in func.blocks:
            new_list = []
            changed = False
            for inst in blk.instructions:
                si = inst.sync_info
                waits = list(si.on_wait) if si is not None else []
                if len(waits) > 1:
                    changed = True
                    for w in waits[:-1]:
                        ev = mybir.InstEventSemaphore(
                            name=f"wsplit-{nc.next_id()}", ins=[], outs=[]
                        )
                        ev.engine = inst.engine
                        ev.sync_info = bass_rust.SyncInfo(on_wait=[w], on_update=[])
                        new_list.append(ev)
                        n_split += 1
                    inst.sync_info = bass_rust.SyncInfo(
                        on_wait=[waits[-1]], on_update=list(si.on_update)
                    )
                new_list.append(inst)
            if changed:
                blk.instructions = new_list
    return n_split


def _build(n_reps=1):
    import concourse.bass as bass
    import concourse.tile as tile
    from concourse import mybir
    from contextlib import ExitStack

    dt = mybir.dt
    f32 = dt.float32
    f32r = dt.float32r
    bf16 = dt.bfloat16
    i32 = dt.int32
    Alu = mybir.AluOpType
    Act = mybir.ActivationFunctionType
    ts = bass.ts

    nc = bass.Bass("TRN2", target_bir_lowering=False, debug=False,
                   num_devices=NCORES)
    x_io = nc.dram_tensor("x", [B, C, NVOX], f32r, kind="ExternalInput").ap()
    y_io = nc.dram_tensor("y", [B, COND, C, NVOX], f32r, kind="ExternalInput").ap()
    wcat_io = nc.dram_tensor("wcat", [C, 4 * C], f32r, kind="ExternalInput").ap()
    vb_io = nc.dram_tensor("vb", [C, 4], f32, kind="ExternalInput").ap()
    out_io = nc.dram_tensor("out", [B, C, NVOX], f32, kind="ExternalOutput").ap()

    def mm(out, lhsT, rhs, start=True, stop=True):
        nc.tensor.matmul(out, lhsT=lhsT, rhs=rhs, start=start, stop=stop)

    with tile.TileContext(nc) as tc, ExitStack() as ctx:
        # ---------------- constants / weights / masks -------------------
        const = ctx.enter_context(tc.tile_pool(name="const", bufs=1))
        dram = ctx.enter_context(tc.tile_pool(name="dram", bufs=1, space="DRAM"))

        wcat = const.tile([C, 4 * C], f32r, tag="wcat")
        nc.sync.dma_start(wcat[:], wcat_io[:])
        vb = const.tile([C, 4], f32, tag="vb")
        nc.sync.dma_start(vb[:], vb_io[:])
        vecs = {name: vb[:, i:i + 1]
                for i, name in enumerate(("bq", "bo2", "gamma", "beta"))}
        wT = {name: wcat[:, ts_w * C:(ts_w + 1) * C]
              for ts_w, name in enumerate(("wq", "wk", "wv", "wo"))}
        bo2 = vecs["bo2"]

        # --- masks via iota + compare (int32), cast on copy
        with tc.tile_pool(name="setup", bufs=1) as setup:
            def icast(dst_ap, src_ap):
                nc.vector.tensor_copy(dst_ap, src_ap)

            p128 = setup.tile([C, C], i32, tag="p128")
            nc.gpsimd.iota(p128[:], pattern=[[0, C]], base=0, channel_multiplier=1)
            f128 = setup.tile([C, C], i32, tag="f128")
            nc.gpsimd.iota(f128[:], pattern=[[1, C]], base=0, channel_multiplier=0)
            hc128 = setup.tile([C, C], i32, tag="hc128")
            nc.vector.tensor_scalar(hc128[:], p128[:], 4, None,
                                    Alu.arith_shift_right)
            tmpi = setup.tile([C, C], i32, tag="tmpi")

            # identity [128,128] (for the residual accumulate matmul)
            ident = const.tile([C, C], f32r, tag="ident")
            nc.vector.tensor_tensor(tmpi[:], f128[:], p128[:], Alu.is_equal)
            icast(ident[:], tmpi[:])

            # mask32 [128, 4*32]: col 32j+m ; 1 iff (m - 8j) == c//16
            jm = setup.tile([C, C], i32, tag="jm")
            nc.gpsimd.iota(jm[:].rearrange("p (j m) -> p j m", j=4),
                           pattern=[[-8, 4], [1, 32]], base=0,
                           channel_multiplier=0)
            mask32 = const.tile([C, C], f32r, tag="mask32")
            nc.vector.tensor_tensor(tmpi[:], jm[:], hc128[:], Alu.is_equal)
            icast(mask32[:], tmpi[:])

            # lhsT32 [32,32]: 1 iff p%8 == m%8  (Z replication matmul), bf16
            p32 = setup.tile([32, 32], i32, tag="p32")
            nc.gpsimd.iota(p32[:], pattern=[[0, 32]], base=0, channel_multiplier=1)
            pm32 = setup.tile([32, 32], i32, tag="pm32")
            nc.vector.tensor_scalar(pm32[:], p32[:], 3, 3,
                                    Alu.arith_shift_right, Alu.arith_shift_left)
            t32 = setup.tile([32, 32], i32, tag="t32")
            nc.vector.tensor_tensor(t32[:], p32[:], pm32[:], Alu.subtract)
            fm32 = setup.tile([32, 32], i32, tag="fm32")
            nc.gpsimd.iota(fm32[:].rearrange("p (j m) -> p j m", j=4),
                           pattern=[[0, 4], [1, 8]], base=0, channel_multiplier=0)
            e32 = setup.tile([32, 32], i32, tag="e32")
            nc.vector.tensor_tensor(e32[:], fm32[:], t32[:], Alu.is_equal)
            lhsT32 = const.tile([32, 32], bf16, tag="lhsT32")
            icast(lhsT32[:], e32[:])

            # maskb [32, 4*128]: col 128j+c ; 1 iff (p - 8j) == c//16, bf16
            pj = setup.tile([32, 4 * C], i32, tag="pj")
            nc.gpsimd.iota(pj[:].rearrange("p (j c) -> p j c", j=4),
                           pattern=[[-8, 4], [0, C]], base=0,
                           channel_multiplier=1)
            fc = setup.tile([32, 4 * C], i32, tag="fc")
            nc.gpsimd.iota(fc[:].rearrange("p (j c) -> p j c", j=4),
                           pattern=[[0, 4], [1, C]], base=0, channel_multiplier=0)
            nc.vector.tensor_scalar(fc[:], fc[:], 4, None, Alu.arith_shift_right)
            eb = setup.tile([32, 4 * C], i32, tag="eb")
            nc.vector.tensor_tensor(eb[:], pj[:], fc[:], Alu.is_equal)
            maskb = const.tile([32, 4 * C], bf16, tag="maskb")
            icast(maskb[:], eb[:])

            # gmask [128, 8]: 1 iff c//16 == g   (GN group reduction)
            g8 = setup.tile([C, 8], i32, tag="g8")
            nc.gpsimd.iota(g8[:], pattern=[[1, 8]], base=0, channel_multiplier=0)
            e8 = setup.tile([C, 8], i32, tag="e8")
            nc.vector.tensor_tensor(e8[:], g8[:], hc128[:, 0:8], Alu.is_equal)
            gmask = const.tile([C, 8], f32, tag="gmask")
            icast(gmask[:], e8[:])

            # gm2 [8, 128]: 1 iff p == c//16    (GN group -> channel bcast)
            p8 = setup.tile([8, C], i32, tag="p8")
            nc.gpsimd.iota(p8[:], pattern=[[0, C]], base=0, channel_multiplier=1)
            fc8 = setup.tile([8, C], i32, tag="fc8")
            nc.gpsimd.iota(fc8[:], pattern=[[1, C]], base=0, channel_multiplier=0)
            nc.vector.tensor_scalar(fc8[:], fc8[:], 4, None, Alu.arith_shift_right)
            e82 = setup.tile([8, C], i32, tag="e82")
            nc.vector.tensor_tensor(e82[:], p8[:], fc8[:], Alu.is_equal)
            gm2 = const.tile([8, C], f32, tag="gm2")
            icast(gm2[:], e82[:])

        # ---------------- main pipeline pools ----------------------------
        p_x = ctx.enter_context(tc.tile_pool(name="p_x", bufs=2))
        p_y = ctx.enter_context(tc.tile_pool(name="p_y", bufs=4))
        p_sb = ctx.enter_context(tc.tile_pool(name="p_sb", bufs=2))
        p_out = ctx.enter_context(tc.tile_pool(name="p_out", bufs=2))
        ps_sq = ctx.enter_context(tc.tile_pool(name="ps_sq", bufs=1, space="PSUM"))
        ps_oz = ctx.enter_context(tc.tile_pool(name="ps_oz", bufs=1, space="PSUM"))
        ps_k = ctx.enter_context(tc.tile_pool(name="ps_k", bufs=1, space="PSUM"))
        ps_v = ctx.enter_context(tc.tile_pool(name="ps_v", bufs=1, space="PSUM"))
        ps_bb = ctx.enter_context(tc.tile_pool(name="ps_bb", bufs=1, space="PSUM"))

        NK = B * NTILES

        for rep in range(n_reps):
            out_acc = p_out.tile([C, B * NVOX], f32, tag="out_acc")
            sums = p_out.tile([C, NK], f32, tag="sums")
            ssqs = p_out.tile([C, NK], f32, tag="ssqs")
            dump = p_out.tile([C, NT], f32, tag="dump")

            xres_b = {}
            ytiles = {}
            st_front = {}
            st_mid = {}
            cc_state = {}
            if rep == 0:
                gn_post_by_rep = {}
                prev_posts = None

            def load_x(b, part, lo, hi):
                if part == 0:
                    xr = p_x.tile([C, NVOX], f32r, tag="xres")
                    xres_b[b] = xr
                xr = xres_b[b]
                nc.sync.dma_start(xr[:, lo:hi], x_io[b][:, lo:hi])

            def load_y(k):
                b, t = k // NTILES, k % NTILES
                yt = p_y.tile([C, COND * NT], f32r, tag="y")
                ysrc = y_io[b].rearrange("j c v -> c j v")
                nc.sync.dma_start(
                    yt[:].rearrange("p (j v) -> p j v", j=COND),
                    ysrc[:, :, ts(t, NT)])
                ytiles[k] = yt

            qsb_of = {}

            def front_q(k):
                b, t = k // NTILES, k % NTILES
                xt = xres_b[b][:, ts(t, NT)]
                psQ = ps_sq.tile([C, NT], f32, tag="sq")
                mm(psQ[:], wT["wq"][:], xt)
                qsb = p_sb.tile([C, NT], f32, tag="qsb", bufs=4)
                nc.scalar.activation(qsb[:], psQ[:], Act.Identity,
                                     bias=vecs["bq"])
                qsb_of[k] = (xt, qsb)

            def front_a(k):
                xt, qsb = qsb_of.pop(k)
                qkbig = p_sb.tile([C, COND * NT], f32r, tag="qkbig")
                st_front[k] = {"xt": xt, "qsb": qsb, "qkbig": qkbig}

            def front_k(k, h):
                """K projections for conds 2h, 2h+1 into one 2-bank tile."""
                yt = ytiles[k]
                psk = ps_k.tile([C, 2 * NT], f32, tag="k01")
                for i, j in enumerate((2 * h, 2 * h + 1)):
                    mm(psk[:, ts(i, NT)], wT["wk"][:], yt[:, ts(j, NT)])
                st_front[k][f"psk{h}"] = psk

            def front_qk(k, h):
                st = st_front[k]
                qsb, qkbig, psk = st["qsb"], st["qkbig"], st[f"psk{h}"]
                qbc = (qsb[:].rearrange("p (o v) -> p o v", o=1)
                       .broadcast_to([C, 2, NT]))
                nc.vector.tensor_tensor(
                    qkbig[:, ts(h, 2 * NT)].rearrange("p (t v) -> p t v", t=2),
                    psk[:].rearrange("p (t v) -> p t v", t=2),
                    qbc, Alu.mult)

            def front_s(k, h):
                st = st_front[k]
                if h == 0:
                    st["psS"] = ps_sq.tile([32, NT], f32, tag="sq", name="psS")
                qkbig, psS = st["qkbig"], st["psS"]
                for j in (2 * h, 2 * h + 1):
                    mm(psS[:], mask32[:, ts(j, 32)], qkbig[:, ts(j, NT)],
                       start=(j == 0), stop=(j == COND - 1))
                if h == 1:
                    st_front.pop(k)
                    st_mid[k] = (st["xt"], psS)

            def mid(k):
                xt, psS = st_mid[k]
                esb = p_sb.tile([32, NT], bf16, tag="esb")
                nc.scalar.activation(esb[:], psS[:], Act.Exp, scale=0.25)
                psZ = ps_oz.tile([32, NT], f32, tag="oz")
                mm(psZ[:], lhsT32[:], esb[:])
                rsb = p_sb.tile([32, NT], bf16, tag="rsb")
                with nc.allow_low_precision(reason="softmax weights in bf16"):
                    nc.vector.reciprocal(rsb[:], psZ[:])
                etsb = p_sb.tile([32, NT], bf16, tag="etsb", bufs=3)
                nc.gpsimd.tensor_tensor(etsb[:], esb[:], rsb[:], Alu.mult)
                wbig = p_sb.tile([C, COND * NT], f32r, tag="wbig", bufs=3)
                st_mid[k] = (xt, etsb, wbig)

            def back_v(k, h):
                """V pair (moved to SBUF via Act) + attn-bcast pair +
                one [C,2NT] attn*V on DVE (single PSUM operand)."""
                xt, etsb, wbig = st_mid[k]
                yt = ytiles[k] if h == 0 else ytiles.pop(k)
                psV = ps_v.tile([C, 2 * NT], f32, tag="v01")
                psBB = ps_bb.tile([C, 2 * NT], f32, tag="bb01")
                for i, j in enumerate((2 * h, 2 * h + 1)):
                    mm(psV[:, ts(i, NT)], wT["wv"][:], yt[:, ts(j, NT)])
                    mm(psBB[:, ts(i, NT)], maskb[:, ts(j, C)], etsb[:])
                vbig = p_sb.tile([C, 2 * NT], bf16, tag="vbig")
                nc.scalar.copy(vbig[:], psV[:])
                nc.vector.tensor_tensor(wbig[:, ts(h, 2 * NT)], psBB[:],
                                        vbig[:], Alu.mult)

            def back_o(k):
                b, t = k // NTILES, k % NTILES
                col = k
                xt, etsb, wbig = st_mid.pop(k)
                psO = ps_oz.tile([C, NT], f32, tag="oz")
                for j in range(COND):
                    mm(psO[:], wT["wo"][:], wbig[:, ts(j, NT)],
                       start=(j == 0), stop=False)
                mm(psO[:], ident[:], xt, start=False, stop=True)
                outt = out_acc[:, col * NT: (col + 1) * NT]
                nc.scalar.activation(
                    outt, psO[:], Act.Identity, bias=bo2,
                    accum_out=sums[:, col: col + 1])
                nc.scalar.activation(
                    dump[:], outt, Act.Square,
                    accum_out=ssqs[:, col: col + 1])

            def gn_pre(b):
                """Per-channel partials -> DRAM -> AllGather across cores."""
                ccsb = p_out.tile([C, 2], f32, tag=f"ccsb{b}")
                nc.vector.reduce_sum(ccsb[:, 0:1],
                                     sums[:, b * NTILES:(b + 1) * NTILES],
                                     axis=mybir.AxisListType.X)
                nc.vector.reduce_sum(ccsb[:, 1:2],
                                     ssqs[:, b * NTILES:(b + 1) * NTILES],
                                     axis=mybir.AxisListType.X)
                cc_in = dram.tile([C, 2], f32, tag=f"cc_in{b}")
                cc_all = dram.tile([NCORES, C * 2], f32, tag=f"cc_all{b}")
                nc.sync.dma_start(cc_in[:], ccsb[:])
                nc.gpsimd.collective_compute(
                    "AllGather", Alu.bypass,
                    replica_groups=[list(range(NCORES))],
                    ins=[cc_in.opt()], outs=[cc_all.opt()])
                cc_state[b] = cc_all

            def gn_post(b, cc_state=cc_state, out_acc=out_acc):
                """Gathered stats -> affine -> rescale out_acc -> store."""
                cc_all = cc_state.pop(b)
                gsb = p_out.tile([C, 2 * NCORES], f32, tag=f"gsb{b}")
                nc.sync.dma_start(
                    gsb[:].rearrange("c (g s) -> c g s", g=NCORES),
                    cc_all[:].rearrange("g (c s) -> c g s", c=C))
                red = p_out.tile([C, 2], f32, tag=f"red{b}")
                nc.vector.reduce_sum(
                    red[:], gsb[:].rearrange("c (g s) -> c s g", g=NCORES),
                    axis=mybir.AxisListType.X)
                psG = ps_oz.tile([8, 2], f32, tag="oz")
                nc.tensor.matmul(psG[:], lhsT=gmask[:], rhs=red[:],
                                 start=True, stop=True)
                msb = p_out.tile([8, 2], f32, tag=f"msb{b}")
                nc.vector.tensor_scalar(msb[:], psG[:], 1.0 / N_GROUP, None,
                                        Alu.mult)
                vtmp = p_out.tile([8, 2], f32, tag=f"vtmp{b}")
                eps_t = p_out.tile([8, 1], f32, tag=f"eps{b}")
                nc.vector.memset(eps_t[:], EPS)
                nc.vector.tensor_tensor(vtmp[:, 0:1], msb[:, 0:1],
                                        msb[:, 0:1], Alu.mult)
                nc.vector.tensor_tensor(vtmp[:, 1:2], msb[:, 1:2],
                                        vtmp[:, 0:1], Alu.subtract)
                nc.scalar.activation(vtmp[:, 0:1], vtmp[:, 1:2], Act.Sqrt,
                                     bias=eps_t[:])
                pstat = p_out.tile([8, 2], f32, tag=f"pstat{b}")
                nc.vector.tensor_copy(pstat[:, 0:1], msb[:, 0:1])
                nc.vector.reciprocal(pstat[:, 1:2], vtmp[:, 0:1])
                psP = ps_oz.tile([C, 2], f32, tag="oz")
                nc.tensor.matmul(psP[:], lhsT=gm2[:], rhs=pstat[:],
                                 start=True, stop=True)
                scale_b = p_out.tile([C, 1], f32, tag=f"scale{b}")
                nc.vector.tensor_tensor(scale_b[:], psP[:, 1:2],
                                        vecs["gamma"], Alu.mult)
                mscale = p_out.tile([C, 1], f32, tag=f"mscale{b}")
                nc.vector.tensor_tensor(mscale[:], psP[:, 0:1], scale_b[:],
                                        Alu.mult)
                bias_b = p_out.tile([C, 1], f32, tag=f"bias{b}")
                nc.vector.tensor_tensor(bias_b[:], vecs["beta"], mscale[:],
                                        Alu.subtract)
                # reuses the xres buffers (x[b] fully consumed by now)
                # chunks rescaled on Act/DVE/Pool into small staging tiles
                q4 = NVOX // 4
                for ci in range(4):
                    srcc = out_acc[:, b * NVOX + ci * q4:
                                   b * NVOX + (ci + 1) * q4]
                    fin4 = p_sb.tile([C, q4], f32, tag="fin4", bufs=2,
                                     name="fin4")
                    dst = fin4[:]
                    if ci == 1:
                        nc.vector.tensor_scalar(dst, srcc, scale_b[:],
                                                bias_b[:], Alu.mult, Alu.add)
                    elif ci == 0:
                        nc.scalar.activation(dst, srcc, Act.Identity,
                                             scale=scale_b[:], bias=bias_b[:])
                    else:
                        nc.gpsimd.tensor_scalar(dst, srcc, scale_b[:],
                                                bias_b[:], Alu.mult, Alu.add)
                    nc.sync.dma_start(
                        out_io[b][:, ci * q4: (ci + 1) * q4], dst)

            # ---------------- software-pipelined emission ----------------
            load_x(0, 0, 0, NT)
            load_y(0)
            load_x(0, 1, NT, NVOX)
            load_y(1)

            front_q(0)
            front_q(1)
            for s in range(NK + 3):
                if rep > 0 and s == 4:
                    with tc.tile_wait_until(0.4 * rep):
                        prev_posts[0]()
                if rep > 0 and s == 6:
                    with tc.tile_wait_until(0.4 * rep + 0.02):
                        prev_posts[1]()
                if s == 2:
                    load_x(1, 0, 0, NVOX // 2)
                if s == 4:
                    load_x(1, 1, NVOX // 2, NVOX)
                if s + 2 < NK:
                    load_y(s + 2)
                if s < NK:
                    front_a(s)
                if 1 <= s <= NK:
                    mid(s - 1)
                if 2 <= s <= NK + 1:
                    back_v(s - 2, 0)
                if s < NK:
                    front_k(s, 0)
                    front_qk(s, 0)
                    front_k(s, 1)
                    front_s(s, 0)
                if 2 <= s <= NK + 1:
                    back_v(s - 2, 1)
                if s < NK:
                    front_qk(s, 1)
                    front_s(s, 1)
                if s + 2 < NK:
                    front_q(s + 2)
                if 3 <= s <= NK + 2:
                    back_o(s - 3)
                    if (s - 3) % NTILES == NTILES - 1:
                        gn_pre((s - 3) // NTILES)
            # posts are deferred into the next rep's steps so collectives
            # overlap the following rep; the final rep drains at the end.
            prev_posts = (lambda r=rep: gn_post_by_rep[(r, 0)](),
                          lambda r=rep: gn_post_by_rep[(r, 1)]())
            gn_post_by_rep[(rep, 0)] = lambda g=gn_post: g(0)
            gn_post_by_rep[(rep, 1)] = lambda g=gn_post: g(1)
            if rep == n_reps - 1:
                with tc.tile_wait_until(0.4 * n_reps):
                    gn_post(0)
                with tc.tile_wait_until(0.4 * n_reps + 0.02):
                    gn_post(B - 1)

    _split_waits(nc)
    return nc


def _shard_inputs(inputs):
    x = np.ascontiguousarray(np.asarray(inputs["decoder_features"], np.float32))
    y = np.ascontiguousarray(
        np.asarray(inputs["skip_connection_features"], np.float32))
    wo = np.asarray(inputs["w_o"], np.float32)
    bo2 = wo @ np.asarray(inputs["b_v"], np.float32) + \
        np.asarray(inputs["b_o"], np.float32)
    base = {
        "wcat": np.ascontiguousarray(np.concatenate(
            [np.asarray(inputs[n], np.float32).T
             for n in ("w_q", "w_k", "w_v", "w_o")], axis=1)),
        "vb": np.ascontiguousarray(np.stack(
            [np.asarray(inputs["b_q"], np.float32), bo2,
             np.asarray(inputs["gn_gamma"], np.float32),
             np.asarray(inputs["gn_beta"], np.float32)], axis=1)),
    }
    in_maps = []
    for ci in range(NCORES):
        sl = slice(HS * ci, HS * (ci + 1))
        im = dict(base)
        im["x"] = np.ascontiguousarray(x[:, :, sl]).reshape(B, C, NVOX)
        im["y"] = np.ascontiguousarray(y[:, :, :, sl]).reshape(B, COND, C, NVOX)
        in_maps.append(im)
    return in_maps


class _Runner:
    """Persistent PJRT runner: trace/compile once, execute many times.

    Mirrors concourse.bass2jax.run_bass_via_pjrt's multi-core branch but
    keeps the jitted shard_map callable alive so repeat calls skip
    re-tracing and NEFF recompilation.
    """

    def __init__(self, nc, donate=True):
        import jax
        from jax.sharding import Mesh, PartitionSpec
        from jax.experimental.shard_map import shard_map
        from concourse import bass2jax, mybir

        bass2jax.install_neuronx_cc_hook()
        assert nc.dbg_addr is None
        partition_name = (nc.partition_id_tensor.name
                          if nc.partition_id_tensor else None)
        in_names, out_names, out_avals, zero_outs = [], [], [], []
        for alloc in nc.m.functions[0].allocations:
            if not isinstance(alloc, mybir.MemoryLocationSet):
                continue
            name = alloc.memorylocations[0].name
            if alloc.kind == "ExternalInput":
                if name != partition_name:
                    in_names.append(name)
            elif alloc.kind == "ExternalOutput":
                out_names.append(name)
                shape = tuple(alloc.tensor_shape)
                dtype = mybir.dt.np(alloc.dtype)
                out_avals.append(jax.core.ShapedArray(shape, dtype))
                zero_outs.append(np.zeros(shape, dtype))
        n_params = len(in_names)
        n_outs = len(out_avals)
        in_names.extend(out_names)
        if partition_name is not None:
            in_names.append(partition_name)
        donate_idx = tuple(range(n_params, n_params + n_outs)) if donate else ()

        def _body(*args):
            operands = list(args)
            if partition_name is not None:
                operands.append(bass2jax.partition_id_tensor())
            outs = bass2jax._bass_exec_p.bind(
                *operands,
                out_avals=tuple(out_avals),
                in_names=tuple(in_names),
                out_names=tuple(out_names),
                lowering_input_output_aliases=(),
                sim_require_finite=True,
                sim_require_nnan=True,
                nc=nc,
            )
            return tuple(outs)

        devices = jax.devices()[:NCORES]
        mesh = Mesh(np.asarray(devices), ("core",))
        in_specs = (PartitionSpec("core"),) * (n_params + n_outs)
        out_specs = (PartitionSpec("core"),) * n_outs
        self._fn = jax.jit(
            shard_map(_body, mesh=mesh, in_specs=in_specs,
                      out_specs=out_specs, check_rep=False),
            donate_argnums=donate_idx, keep_unused=True)
        self._in_names = in_names[:n_params]
        self._out_names = out_names
        self._out_avals = out_avals
        self._zero_outs = zero_outs
        self._jax = jax

    def __call__(self, in_maps):
        concat_in = [
            np.concatenate([np.asarray(m[name]) for m in in_maps], axis=0)
            for name in self._in_names
        ]
        concat_zeros = [
            np.zeros((NCORES * z.shape[0], *z.shape[1:]), z.dtype)
            for z in self._zero_outs
        ]
        out_arrs = self._fn(*concat_in, *concat_zeros)
        out_arrs = self._jax.block_until_ready(out_arrs)
        return [
            {
                name: np.asarray(out_arrs[i]).reshape(
                    NCORES, *self._out_avals[i].shape)[c]
                for i, name in enumerate(self._out_names)
            }
            for c in range(NCORES)
        ]


class _Results:
    def __init__(self, results):
        self.results = results


def _get_runner(n_reps=1, donate=True):
    key = (n_reps, donate)
    if key not in _CACHE:
        _CACHE[key] = _Runner(_build(n_reps), donate=donate)
    return _CACHE[key]


def _run(in_maps, n_reps=1):
    return _Results(_get_runner(n_reps)(in_maps))


def kernel(**inputs) -> np.ndarray:
    res = _run(_shard_inputs(inputs))
    out = np.empty((B, C, H, W, D), np.float32)
    for ci in range(NCORES):
        sl = slice(HS * ci, HS * (ci + 1))
        out[:, :, sl] = res.results[ci]["out"].reshape(B, C, HS, W, D)
    return out


# revision 40
# speedup vs baseline: 1.0355x; 1.0355x over previous
"""Trainium2 Bass kernel for nn_DecoderCrossAttention.

Reference computation (per voxel v, batch b):
    q = Wq x_v + bq                        (x = decoder_features, [C])
    k_j = Wk y_jv + bk, v_j = Wv y_jv + bv (y = skip features, COND=4 frames)
    s_j[h] = <q_h, k_jh> / sqrt(DH)        (NH=8 heads of DH=16)
    attn = softmax_j(s)                    (over the 4 conditioning frames)
    o = Wo (sum_j attn_j * v_j) + bo + x_v
    out = GroupNorm8(o) * gamma + beta     (stats over (C/G, H, W, D) per batch)

Algebraic folds:
  * bk shifts all 4 logits of each softmax equally -> dropped entirely.
  * sum_j attn_j == 1, so bv contributes Wo bv to every voxel ->
    folded into bo2 = Wo bv + bo (computed once on device).

Strategy (8 NeuronCores, data-parallel over H):
  * Each core gets H-slice of 4 planes: 2*4*32*32 = 8192 voxels.
  * Feature-major layout [C=128 partitions, voxels in free dim], 512-voxel
    tiles, 4-deep software pipeline (front / softmax / attn-apply / out).
  * All projections are PE matmuls in float32r (full rate at N=512).
  * Per-head score reduction and the softmax broadcast (8 head rows -> 128
    channels) are PE matmuls against 0/1 masks built in-kernel.
  * Softmax tail runs in bf16 (exp out, 1/Z, attn weights, broadcast
    matmul) - masks are exact 0/1 in bf16, 2x DVE rate on etsb.
  * V parks in PSUM; attn*V is a Pool tensor_tensor reading both operands
    from PSUM - no PSUM->SBUF moves for V at all.
  * Residual + bo2 + per-channel GN sums fused in one DVE stt; squares
    accumulated on Act.
  * GroupNorm stats cross over cores via AllGather (15us) + local strided
    reduce instead of AllReduce (28us); GN rescale is Act Identity with
    per-partition scale/bias vectors.

The walrus build here accepts only ONE sync wait per instruction; Tile
attaches many.  split_waits() hoists extras onto standalone EventSemaphore
instructions post-scheduling.
"""

import sys

if "/opt/trn_rl_repo" not in sys.path:
    sys.path.insert(0, "/opt/trn_rl_repo")

import numpy as np

B, COND, C, H, W, D = 2, 4, 128, 32, 32, 32
NH, DH, G = 8, 16, 8
EPS = 1e-5
NCORES = 8
HS = H // NCORES          # 4 H-planes per core
NVOX = HS * W * D         # 4096 voxels per batch per core
NT = 512                  # voxels per tile
NTILES = NVOX // NT       # 8 tiles per batch
N_GROUP = (C // G) * H * W * D   # elements per (batch, group) for GN stats

_CACHE = {}


def _split_waits(nc):
    """Hoist extra sync waits onto standalone EventSemaphore instructions."""
    from concourse import mybir
    import bass_rust

    n_split = 0
    for func in nc.m.functions:
        for blk in func.blocks:
            new_list = []
            changed = False
            for inst in blk.instructions:
                si = inst.sync_info
                waits = list(si.on_wait) if si is not None else []
                if len(waits) > 1:
                    changed = True
                    for w in waits[:-1]:
                        ev = mybir.InstEventSemaphore(
                            name=f"wsplit-{nc.next_id()}", ins=[], outs=[]
                        )
                        ev.engine = inst.engine
                        ev.sync_info = bass_rust.SyncInfo(on_wait=[w], on_update=[])
                        new_list.append(ev)
                        n_split += 1
                    inst.sync_info = bass_rust.SyncInfo(
                        on_wait=[waits[-1]], on_update=list(si.on_update)
                    )
                new_list.append(inst)
            if changed:
                blk.instructions = new_list
    return n_split


def _build(n_reps=1):
    import concourse.bass as bass
    import concourse.tile as tile
    from concourse import mybir
    from contextlib import ExitStack

    dt = mybir.dt
    f32 = dt.float32
    f32r = dt.float32r
    bf16 = dt.bfloat16
    i32 = dt.int32
    Alu = mybir.AluOpType
    Act = mybir.ActivationFunctionType
    ts = bass.ts

    nc = bass.Bass("TRN2", target_bir_lowering=False, debug=False,
                   num_devices=NCORES)
    x_io = nc.dram_tensor("x", [B, C, NVOX], f32r, kind="ExternalInput").ap()
    y_io = nc.dram_tensor("y", [B, COND, C, NVOX], f32r, kind="ExternalInput").ap()
    wcat_io = nc.dram_tensor("wcat", [C, 4 * C], f32r, kind="ExternalInput").ap()
    vb_io = nc.dram_tensor("vb", [C, 4], f32, kind="ExternalInput").ap()
    out_io = nc.dram_tensor("out", [B, C, NVOX], f32, kind="ExternalOutput").ap()

    def mm(out, lhsT, rhs, start=True, stop=True):
        nc.tensor.matmul(out, lhsT=lhsT, rhs=rhs, start=start, stop=stop)

    with tile.TileContext(nc) as tc, ExitStack() as ctx:
        # ---------------- constants / weights / masks -------------------
        const = ctx.enter_context(tc.tile_pool(name="const", bufs=1))
        dram = ctx.enter_context(tc.tile_pool(name="dram", bufs=1, space="DRAM"))

        wcat = const.tile([C, 4 * C], f32r, tag="wcat")
        nc.sync.dma_start(wcat[:], wcat_io[:])
        vb = const.tile([C, 4], f32, tag="vb")
        nc.sync.dma_start(vb[:], vb_io[:])
        vecs = {name: vb[:, i:i + 1]
                for i, name in enumerate(("bq", "bo2", "gamma", "beta"))}
        wT = {name: wcat[:, ts_w * C:(ts_w + 1) * C]
              for ts_w, name in enumerate(("wq", "wk", "wv", "wo"))}
        bo2 = vecs["bo2"]

        # --- masks via iota + compare (int32), cast on copy
        with tc.tile_pool(name="setup", bufs=1) as setup:
            def icast(dst_ap, src_ap):
                nc.vector.tensor_copy(dst_ap, src_ap)

            p128 = setup.tile([C, C], i32, tag="p128")
            nc.gpsimd.iota(p128[:], pattern=[[0, C]], base=0, channel_multiplier=1)
            f128 = setup.tile([C, C], i32, tag="f128")
            nc.gpsimd.iota(f128[:], pattern=[[1, C]], base=0, channel_multiplier=0)
            hc128 = setup.tile([C, C], i32, tag="hc128")
            nc.vector.tensor_scalar(hc128[:], p128[:], 4, None,
                                    Alu.arith_shift_right)
            tmpi = setup.tile([C, C], i32, tag="tmpi")

            # identity [128,128] (for the residual accumulate matmul)
            ident = const.tile([C, C], f32r, tag="ident")
            nc.vector.tensor_tensor(tmpi[:], f128[:], p128[:], Alu.is_equal)
            icast(ident[:], tmpi[:])

            # mask32 [128, 4*32]: col 32j+m ; 1 iff (m - 8j) == c//16
            jm = setup.tile([C, C], i32, tag="jm")
            nc.gpsimd.iota(jm[:].rearrange("p (j m) -> p j m", j=4),
                           pattern=[[-8, 4], [1, 32]], base=0,
                           channel_multiplier=0)
            mask32 = const.tile([C, C], f32r, tag="mask32")
            nc.vector.tensor_tensor(tmpi[:], jm[:], hc128[:], Alu.is_equal)
            icast(mask32[:], tmpi[:])

            # lhsT32 [32,32]: 1 iff p%8 == m%8  (Z replication matmul), bf16
            p32 = setup.tile([32, 32], i32, tag="p32")
            nc.gpsimd.iota(p32[:], pattern=[[0, 32]], base=0, channel_multiplier=1)
            pm32 = setup.tile([32, 32], i32, tag="pm32")
            nc.vector.tensor_scalar(pm32[:], p32[:], 3, 3,
                                    Alu.arith_shift_right, Alu.arith_shift_left)
            t32 = setup.tile([32, 32], i32, tag="t32")
            nc.vector.tensor_tensor(t32[:], p32[:], pm32[:], Alu.subtract)
            fm32 = setup.tile([32, 32], i32, tag="fm32")
            nc.gpsimd.iota(fm32[:].rearrange("p (j m) -> p j m", j=4),
                           pattern=[[0, 4], [1, 8]], base=0, channel_multiplier=0)
            e32 = setup.tile([32, 32], i32, tag="e32")
            nc.vector.tensor_tensor(e32[:], fm32[:], t32[:], Alu.is_equal)
            lhsT32 = const.tile([32, 32], bf16, tag="lhsT32")
            icast(lhsT32[:], e32[:])

            # maskb [32, 4*128]: col 128j+c ; 1 iff (p - 8j) == c//16, bf16
            pj = setup.tile([32, 4 * C], i32, tag="pj")
            nc.gpsimd.iota(pj[:].rearrange("p (j c) -> p j c", j=4),
                           pattern=[[-8, 4], [0, C]], base=0,
                           channel_multiplier=1)
            fc = setup.tile([32, 4 * C], i32, tag="fc")
            nc.gpsimd.iota(fc[:].rearrange("p (j c) -> p j c", j=4),
                           pattern=[[0, 4], [1, C]], base=0, channel_multiplier=0)
            nc.vector.tensor_scalar(fc[:], fc[:], 4, None, Alu.arith_shift_right)
            eb = setup.tile([32, 4 * C], i32, tag="eb")
            nc.vector.tensor_tensor(eb[:], pj[:], fc[:], Alu.is_equal)
            maskb = const.tile([32, 4 * C], bf16, tag="maskb")
            icast(maskb[:], eb[:])

            # gmask [128, 8]: 1 iff c//16 == g   (GN group reduction)
            g8 = setup.tile([C, 8], i32, tag="g8")
            nc.gpsimd.iota(g8[:], pattern=[[1, 8]], base=0, channel_multiplier=0)
            e8 = setup.tile([C, 8], i32, tag="e8")
            nc.vector.tensor_tensor(e8[:], g8[:], hc128[:, 0:8], Alu.is_equal)
            gmask = const.tile([C, 8], f32, tag="gmask")
            icast(gmask[:], e8[:])

            # gm2 [8, 128]: 1 iff p == c//16    (GN group -> channel bcast)
            p8 = setup.tile([8, C], i32, tag="p8")
            nc.gpsimd.iota(p8[:], pattern=[[0, C]], base=0, channel_multiplier=1)
            fc8 = setup.tile([8, C], i32, tag="fc8")
            nc.gpsimd.iota(fc8[:], pattern=[[1, C]], base=0, channel_multiplier=0)
            nc.vector.tensor_scalar(fc8[:], fc8[:], 4, None, Alu.arith_shift_right)
            e82 = setup.tile([8, C], i32, tag="e82")
            nc.vector.tensor_tensor(e82[:], p8[:], fc8[:], Alu.is_equal)
            gm2 = const.tile([8, C], f32, tag="gm2")
            icast(gm2[:], e82[:])

        # ---------------- main pipeline pools ----------------------------
        p_x = ctx.enter_context(tc.tile_pool(name="p_x", bufs=2))
        p_y = ctx.enter_context(tc.tile_pool(name="p_y", bufs=4))
        p_sb = ctx.enter_context(tc.tile_pool(name="p_sb", bufs=2))
        p_out = ctx.enter_context(tc.tile_pool(name="p_out", bufs=2))
        ps_sq = ctx.enter_context(tc.tile_pool(name="ps_sq", bufs=1, space="PSUM"))
        ps_oz = ctx.enter_context(tc.tile_pool(name="ps_oz", bufs=1, space="PSUM"))
        ps_k = ctx.enter_context(tc.tile_pool(name="ps_k", bufs=1, space="PSUM"))
        ps_v = ctx.enter_context(tc.tile_pool(name="ps_v", bufs=1, space="PSUM"))
        ps_bb = ctx.enter_context(tc.tile_pool(name="ps_bb", bufs=1, space="PSUM"))

        NK = B * NTILES

        for rep in range(n_reps):
            out_acc = p_out.tile([C, B * NVOX], f32, tag="out_acc")
            sums = p_out.tile([C, NK], f32, tag="sums")
            ssqs = p_out.tile([C, NK], f32, tag="ssqs")
            dump = p_out.tile([C, NT], f32, tag="dump")

            xres_b = {}
            ytiles = {}
            st_front = {}
            st_mid = {}
            cc_state = {}
            if rep == 0:
                gn_post_by_rep = {}
                prev_posts = None

            def load_x(b, part, lo, hi):
                if part == 0:
                    xr = p_x.tile([C, NVOX], f32r, tag="xres")
                    xres_b[b] = xr
                xr = xres_b[b]
                nc.sync.dma_start(xr[:, lo:hi], x_io[b][:, lo:hi])

            def load_y(k):
                b, t = k // NTILES, k % NTILES
                yt = p_y.tile([C, COND * NT], f32r, tag="y")
                ysrc = y_io[b].rearrange("j c v -> c j v")
                nc.sync.dma_start(
                    yt[:].rearrange("p (j v) -> p j v", j=COND),
                    ysrc[:, :, ts(t, NT)])
                ytiles[k] = yt

            qsb_of = {}

            def front_q(k):
                b, t = k // NTILES, k % NTILES
                xt = xres_b[b][:, ts(t, NT)]
                psQ = ps_sq.tile([C, NT], f32, tag="sq")
                mm(psQ[:], wT["wq"][:], xt)
                qsb = p_sb.tile([C, NT], f32, tag="qsb", bufs=4)
                nc.scalar.activation(qsb[:], psQ[:], Act.Identity,
                                     bias=vecs["bq"])
                qsb_of[k] = (xt, qsb)

            def front_a(k):
                xt, qsb = qsb_of.pop(k)
                qkbig = p_sb.tile([C, COND * NT], f32r, tag="qkbig")
                st_front[k] = {"xt": xt, "qsb": qsb, "qkbig": qkbig}

            def front_k(k, h):
                """K projections for conds 2h, 2h+1 into one 2-bank tile."""
                yt = ytiles[k]
                psk = ps_k.tile([C, 2 * NT], f32, tag="k01")
                for i, j in enumerate((2 * h, 2 * h + 1)):
                    mm(psk[:, ts(i, NT)], wT["wk"][:], yt[:, ts(j, NT)])
                st_front[k][f"psk{h}"] = psk

            def front_qk(k, h):
                st = st_front[k]
                qsb, qkbig, psk = st["qsb"], st["qkbig"], st[f"psk{h}"]
                qbc = (qsb[:].rearrange("p (o v) -> p o v", o=1)
                       .broadcast_to([C, 2, NT]))
                nc.vector.tensor_tensor(
                    qkbig[:, ts(h, 2 * NT)].rearrange("p (t v) -> p t v", t=2),
                    psk[:].rearrange("p (t v) -> p t v", t=2),
                    qbc, Alu.mult)

            def front_s(k, h):
                st = st_front[k]
                if h == 0:
                    st["psS"] = ps_sq.tile([32, NT], f32, tag="sq", name="psS")
                qkbig, psS = st["qkbig"], st["psS"]
                for j in (2 * h, 2 * h + 1):
                    mm(psS[:], mask32[:, ts(j, 32)], qkbig[:, ts(j, NT)],
                       start=(j == 0), stop=(j == COND - 1))
                if h == 1:
                    st_front.pop(k)
                    st_mid[k] = (st["xt"], psS)

            def mid(k):
                xt, psS = st_mid[k]
                esb = p_sb.tile([32, NT], bf16, tag="esb")
                nc.scalar.activation(esb[:], psS[:], Act.Exp, scale=0.25)
                psZ = ps_oz.tile([32, NT], f32, tag="oz")
                mm(psZ[:], lhsT32[:], esb[:])
                rsb = p_sb.tile([32, NT], bf16, tag="rsb")
                with nc.allow_low_precision(reason="softmax weights in bf16"):
                    nc.vector.reciprocal(rsb[:], psZ[:])
                etsb = p_sb.tile([32, NT], bf16, tag="etsb", bufs=3)
                nc.gpsimd.tensor_tensor(etsb[:], esb[:], rsb[:], Alu.mult)
                wbig = p_sb.tile([C, COND * NT], f32r, tag="wbig", bufs=3)
                st_mid[k] = (xt, etsb, wbig)

            def back_v(k, h):
                """V pair (moved to SBUF via Act) + attn-bcast pair +
                one [C,2NT] attn*V on DVE (single PSUM operand)."""
                xt, etsb, wbig = st_mid[k]
                yt = ytiles[k] if h == 0 else ytiles.pop(k)
                psV = ps_v.tile([C, 2 * NT], f32, tag="v01")
                psBB = ps_bb.tile([C, 2 * NT], f32, tag="bb01")
                for i, j in enumerate((2 * h, 2 * h + 1)):
                    mm(psV[:, ts(i, NT)], wT["wv"][:], yt[:, ts(j, NT)])
                    mm(psBB[:, ts(i, NT)], maskb[:, ts(j, C)], etsb[:])
                vbig = p_sb.tile([C, 2 * NT], bf16, tag="vbig")
                nc.scalar.copy(vbig[:], psV[:])
                nc.vector.tensor_tensor(wbig[:, ts(h, 2 * NT)], psBB[:],
                                        vbig[:], Alu.mult)

            def back_o(k):
                b, t = k // NTILES, k % NTILES
                col = k
                xt, etsb, wbig = st_mid.pop(k)
                psO = ps_oz.tile([C, NT], f32, tag="oz")
                for j in range(COND):
                    mm(psO[:], wT["wo"][:], wbig[:, ts(j, NT)],
                       start=(j == 0), stop=False)
                mm(psO[:], ident[:], xt, start=False, stop=True)
                outt = out_acc[:, col * NT: (col + 1) * NT]
                nc.scalar.activation(
                    outt, psO[:], Act.Identity, bias=bo2,
                    accum_out=sums[:, col: col + 1])
                nc.scalar.activation(
                    dump[:], outt, Act.Square,
                    accum_out=ssqs[:, col: col + 1])

            def gn_pre(b):
                """Per-channel partials -> DRAM -> AllGather across cores."""
                ccsb = p_out.tile([C, 2], f32, tag=f"ccsb{b}")
                nc.vector.reduce_sum(ccsb[:, 0:1],
                                     sums[:, b * NTILES:(b + 1) * NTILES],
                                     axis=mybir.AxisListType.X)
                nc.vector.reduce_sum(ccsb[:, 1:2],
                                     ssqs[:, b * NTILES:(b + 1) * NTILES],
                                     axis=mybir.AxisListType.X)
                cc_in = dram.tile([C, 2], f32, tag=f"cc_in{b}")
                cc_out = dram.tile([C, 2], f32, tag=f"cc_out{b}")
                nc.sync.dma_start(cc_in[:], ccsb[:])
                nc.gpsimd.collective_compute(
                    "AllReduce", Alu.add,
                    replica_groups=[list(range(NCORES))],
                    ins=[cc_in.opt()], outs=[cc_out.opt()])
                cc_state[b] = cc_out

            def gn_post(b, cc_state=cc_state, out_acc=out_acc):
                """Gathered stats -> affine -> rescale out_acc -> store."""
                cc_out = cc_state.pop(b)
                red = p_out.tile([C, 2], f32, tag=f"red{b}")
                nc.sync.dma_start(red[:], cc_out[:])
                psG = ps_oz.tile([8, 2], f32, tag="oz")
                nc.tensor.matmul(psG[:], lhsT=gmask[:], rhs=red[:],
                                 start=True, stop=True)
                msb = p_out.tile([8, 2], f32, tag=f"msb{b}")
                nc.vector.tensor_scalar(msb[:], psG[:], 1.0 / N_GROUP, None,
                                        Alu.mult)
                vtmp = p_out.tile([8, 2], f32, tag=f"vtmp{b}")
                eps_t = p_out.tile([8, 1], f32, tag=f"eps{b}")
                nc.vector.memset(eps_t[:], EPS)
                nc.vector.tensor_tensor(vtmp[:, 0:1], msb[:, 0:1],
                                        msb[:, 0:1], Alu.mult)
                nc.vector.tensor_tensor(vtmp[:, 1:2], msb[:, 1:2],
                                        vtmp[:, 0:1], Alu.subtract)
                nc.scalar.activation(vtmp[:, 0:1], vtmp[:, 1:2], Act.Sqrt,
                                     bias=eps_t[:])
                pstat = p_out.tile([8, 2], f32, tag=f"pstat{b}")
                nc.vector.tensor_copy(pstat[:, 0:1], msb[:, 0:1])
                nc.vector.reciprocal(pstat[:, 1:2], vtmp[:, 0:1])
                psP = ps_oz.tile([C, 2], f32, tag="oz")
                nc.tensor.matmul(psP[:], lhsT=gm2[:], rhs=pstat[:],
                                 start=True, stop=True)
                scale_b = p_out.tile([C, 1], f32, tag=f"scale{b}")
                nc.vector.tensor_tensor(scale_b[:], psP[:, 1:2],
                                        vecs["gamma"], Alu.mult)
                mscale = p_out.tile([C, 1], f32, tag=f"mscale{b}")
                nc.vector.tensor_tensor(mscale[:], psP[:, 0:1], scale_b[:],
                                        Alu.mult)
                bias_b = p_out.tile([C, 1], f32, tag=f"bias{b}")
                nc.vector.tensor_tensor(bias_b[:], vecs["beta"], mscale[:],
                                        Alu.subtract)
                # reuses the xres buffers (x[b] fully consumed by now)
                # chunks rescaled on Act/DVE/Pool into small staging tiles
                q4 = NVOX // 4
                for ci in range(4):
                    srcc = out_acc[:, b * NVOX + ci * q4:
                                   b * NVOX + (ci + 1) * q4]
                    fin4 = p_sb.tile([C, q4], f32, tag="fin4", bufs=2,
                                     name="fin4")
                    dst = fin4[:]
                    if ci == 1:
                        nc.vector.tensor_scalar(dst, srcc, scale_b[:],
                                                bias_b[:], Alu.mult, Alu.add)
                    elif ci == 3:
                        nc.gpsimd.tensor_scalar(dst, srcc, scale_b[:],
                                                bias_b[:], Alu.mult, Alu.add)
                    else:
                        nc.scalar.activation(dst, srcc, Act.Identity,
                                             scale=scale_b[:], bias=bias_b[:])
                    nc.sync.dma_start(
                        out_io[b][:, ci * q4: (ci + 1) * q4], dst)

            # ---------------- software-pipelined emission ----------------
            load_x(0, 0, 0, NT)
            load_y(0)
            load_x(0, 1, NT, NVOX)
            load_y(1)

            front_q(0)
            front_q(1)
            for s in range(NK + 3):
                if rep > 0 and s == 4:
                    with tc.tile_wait_until(0.4 * rep):
                        prev_posts[0]()
                if rep > 0 and s == 6:
                    with tc.tile_wait_until(0.4 * rep + 0.02):
                        prev_posts[1]()
                if s == 2:
                    load_x(1, 0, 0, NVOX // 2)
                if s == 4:
                    load_x(1, 1, NVOX // 2, NVOX)
                if s + 2 < NK:
                    load_y(s + 2)
                if s < NK:
                    front_a(s)
                if 1 <= s <= NK:
                    mid(s - 1)
                if 2 <= s <= NK + 1:
                    back_v(s - 2, 0)
                if s < NK:
                    front_k(s, 0)
                    front_qk(s, 0)
                    front_k(s, 1)
                    front_s(s, 0)
                if 2 <= s <= NK + 1:
                    back_v(s - 2, 1)
                if s < NK:
                    front_qk(s, 1)
                    front_s(s, 1)
                if s + 2 < NK:
                    front_q(s + 2)
                if 3 <= s <= NK + 2:
                    back_o(s - 3)
                    if (s - 3) % NTILES == NTILES - 1:
                        gn_pre((s - 3) // NTILES)
            # posts are deferred into the next rep's steps so collectives
            # overlap the following rep; the final rep drains at the end.
            prev_posts = (lambda r=rep: gn_post_by_rep[(r, 0)](),
                          lambda r=rep: gn_post_by_rep[(r, 1)]())
            gn_post_by_rep[(rep, 0)] = lambda g=gn_post: g(0)
            gn_post_by_rep[(rep, 1)] = lambda g=gn_post: g(1)
            if rep == n_reps - 1:
                with tc.tile_wait_until(0.4 * n_reps):
                    gn_post(0)
                with tc.tile_wait_until(0.4 * n_reps + 0.02):
                    gn_post(B - 1)

    _split_waits(nc)
    return nc


def _shard_inputs(inputs):
    x = np.ascontiguousarray(np.asarray(inputs["decoder_features"], np.float32))
    y = np.ascontiguousarray(
        np.asarray(inputs["skip_connection_features"], np.float32))
    wo = np.asarray(inputs["w_o"], np.float32)
    bo2 = wo @ np.asarray(inputs["b_v"], np.float32) + \
        np.asarray(inputs["b_o"], np.float32)
    base = {
        "wcat": np.ascontiguousarray(np.concatenate(
            [np.asarray(inputs[n], np.float32).T
             for n in ("w_q", "w_k", "w_v", "w_o")], axis=1)),
        "vb": np.ascontiguousarray(np.stack(
            [np.asarray(inputs["b_q"], np.float32), bo2,
             np.asarray(inputs["gn_gamma"], np.float32),
             np.asarray(inputs["gn_beta"], np.float32)], axis=1)),
    }
    in_maps = []
    for ci in range(NCORES):
        sl = slice(HS * ci, HS * (ci + 1))
        im = dict(base)
        im["x"] = np.ascontiguousarray(x[:, :, sl]).reshape(B, C, NVOX)
        im["y"] = np.ascontiguousarray(y[:, :, :, sl]).reshape(B, COND, C, NVOX)
        in_maps.append(im)
    return in_maps


class _Runner:
    """Persistent PJRT runner: trace/compile once, execute many times.

    Mirrors concourse.bass2jax.run_bass_via_pjrt's multi-core branch but
    keeps the jitted shard_map callable alive so repeat calls skip
    re-tracing and NEFF recompilation.
    """

    def __init__(self, nc, donate=True):
        import jax
        from jax.sharding import Mesh, PartitionSpec
        from jax.experimental.shard_map import shard_map
        from concourse import bass2jax, mybir

        bass2jax.install_neuronx_cc_hook()
        assert nc.dbg_addr is None
        partition_name = (nc.partition_id_tensor.name
                          if nc.partition_id_tensor else None)
        in_names, out_names, out_avals, zero_outs = [], [], [], []
        for alloc in nc.m.functions[0].allocations:
            if not isinstance(alloc, mybir.MemoryLocationSet):
                continue
            name = alloc.memorylocations[0].name
            if alloc.kind == "ExternalInput":
                if name != partition_name:
                    in_names.append(name)
            elif alloc.kind == "ExternalOutput":
                out_names.append(name)
                shape = tuple(alloc.tensor_shape)
                dtype = mybir.dt.np(alloc.dtype)
                out_avals.append(jax.core.ShapedArray(shape, dtype))
                zero_outs.append(np.zeros(shape, dtype))
        n_params = len(in_names)
        n_outs = len(out_avals)
        in_names.extend(out_names)
        if partition_name is not None:
            in_names.append(partition_name)
        donate_idx = tuple(range(n_params, n_params + n_outs)) if donate else ()

        def _body(*args):
            operands = list(args)
            if partition_name is not None:
                operands.append(bass2jax.partition_id_tensor())
            outs = bass2jax._bass_exec_p.bind(
                *operands,
                out_avals=tuple(out_avals),
                in_names=tuple(in_names),
                out_names=tuple(out_names),
                lowering_input_output_aliases=(),
                sim_require_finite=True,
                sim_require_nnan=True,
                nc=nc,
            )
            return tuple(outs)

        devices = jax.devices()[:NCORES]
        mesh = Mesh(np.asarray(devices), ("core",))
        in_specs = (PartitionSpec("core"),) * (n_params + n_outs)
        out_specs = (PartitionSpec("core"),) * n_outs
        self._fn = jax.jit(
            shard_map(_body, mesh=mesh, in_specs=in_specs,
                      out_specs=out_specs, check_rep=False),
            donate_argnums=donate_idx, keep_unused=True)
        self._in_names = in_names[:n_params]
        self._out_names = out_names
        self._out_avals = out_avals
        self._zero_outs = zero_outs
        self._jax = jax

    def __call__(self, in_maps):
        concat_in = [
            np.concatenate([np.asarray(m[name]) for m in in_maps], axis=0)
            for name in self._in_names
        ]
        concat_zeros = [
            np.zeros((NCORES * z.shape[0], *z.shape[1:]), z.dtype)
            for z in self._zero_outs
        ]
        out_arrs = self._fn(*concat_in, *concat_zeros)
        out_arrs = self._jax.block_until_ready(out_arrs)
        return [
            {
                name: np.asarray(out_arrs[i]).reshape(
                    NCORES, *self._out_avals[i].shape)[c]
                for i, name in enumerate(self._out_names)
            }
            for c in range(NCORES)
        ]


class _Results:
    def __init__(self, results):
        self.results = results


def _get_runner(n_reps=1, donate=True):
    key = (n_reps, donate)
    if key not in _CACHE:
        _CACHE[key] = _Runner(_build(n_reps), donate=donate)
    return _CACHE[key]


def _run(in_maps, n_reps=1):
    return _Results(_get_runner(n_reps)(in_maps))


def kernel(**inputs) -> np.ndarray:
    res = _run(_shard_inputs(inputs))
    out = np.empty((B, C, H, W, D), np.float32)
    for ci in range(NCORES):
        sl = slice(HS * ci, HS * (ci + 1))
        out[:, :, sl] = res.results[ci]["out"].reshape(B, C, HS, W, D)
    return out
